# revision 3
# baseline (speedup 1.0000x reference)
import sys

sys.path.insert(0, "/opt/trn_rl_repo")
import numpy as np

_CTX = {}

B = 8
N0, N1, N2, N3 = 8192, 2048, 512, 128
EPS = 1e-8


def _build():
    from concourse import bacc, tile
    import concourse.mybir as mybir

    f32 = mybir.dt.float32
    i16 = mybir.dt.int16
    u16 = mybir.dt.uint16
    AF = mybir.ActivationFunctionType
    OP = mybir.AluOpType
    AX = mybir.AxisListType

    nc = bacc.Bacc("TRN2", target_bir_lowering=False, debug=False, num_devices=8)

    qt0_d = nc.dram_tensor("qt0", [4, N0], f32, kind="ExternalInput")
    kt0_d = nc.dram_tensor("kt0", [4, N1], f32, kind="ExternalInput")
    us0_d = nc.dram_tensor("us0", [128, 64], f32, kind="ExternalInput")
    qt1_d = nc.dram_tensor("qt1", [4, N1], f32, kind="ExternalInput")
    kt1_d = nc.dram_tensor("kt1", [4, N2], f32, kind="ExternalInput")
    us1_d = nc.dram_tensor("us1", [128, 16], f32, kind="ExternalInput")
    qt2_d = nc.dram_tensor("qt2", [4, N2], f32, kind="ExternalInput")
    kt2_d = nc.dram_tensor("kt2", [4, N3], f32, kind="ExternalInput")
    us2_d = nc.dram_tensor("us2", [128, 4], f32, kind="ExternalInput")
    f0_d = nc.dram_tensor("f0", [64, N0], f32, kind="ExternalInput")
    f1_d = nc.dram_tensor("f1", [128, N1], f32, kind="ExternalInput")
    f2_d = nc.dram_tensor("f2", [256, N2], f32, kind="ExternalInput")
    f3pm_d = nc.dram_tensor("f3pm", [N3, 512], f32, kind="ExternalInput")
    w00_d = nc.dram_tensor("w00", [192, 128], f32, kind="ExternalInput")
    w01_d = nc.dram_tensor("w01", [128, 128], f32, kind="ExternalInput")
    w10_d = nc.dram_tensor("w10", [384, 256], f32, kind="ExternalInput")
    w11_d = nc.dram_tensor("w11", [256, 128], f32, kind="ExternalInput")
    w20_d = nc.dram_tensor("w20", [768, 256], f32, kind="ExternalInput")
    w21_d = nc.dram_tensor("w21", [256, 256], f32, kind="ExternalInput")
    gb_d = {}
    for nm, nh in [("g00", 1), ("b00", 1), ("g01", 1), ("b01", 1),
                   ("g10", 2), ("b10", 2), ("g11", 1), ("b11", 1),
                   ("g20", 2), ("b20", 2), ("g21", 2), ("b21", 2)]:
        gb_d[nm] = nc.dram_tensor(nm, [128, nh], f32, kind="ExternalInput")
    out_d = nc.dram_tensor("out", [128, N0], f32, kind="ExternalOutput")

    f2pm_d = nc.dram_tensor("f2pm", [N2, 256], f32, kind="Internal")
    f1pm_d = nc.dram_tensor("f1pm", [N1, 128], f32, kind="Internal")
    wrap_ds = {
        "s2": nc.dram_tensor("wrap2", [16, 24 * 4], i16, kind="Internal"),
        "s1": nc.dram_tensor("wrap1", [16, 24 * 16], i16, kind="Internal"),
        "s0": nc.dram_tensor("wrap0", [16, 24 * 64], i16, kind="Internal"),
    }
    ident_d = nc.inline_tensor(np.eye(128, dtype=np.float32), "identnp")

    with tile.TileContext(nc) as tc:
        with tc.tile_pool(name="glob", bufs=1) as gp, \
             tc.tile_pool(name="scr", bufs=1) as sc, \
             tc.tile_pool(name="ps", space="PSUM", bufs=1) as pp, \
             tc.tile_pool(name="dr", space="DRAM", bufs=1) as dp:

            def dma(dst, src):
                nc.sync.dma_start(dst, src)

            ident = gp.tile([128, 128], f32, name="ident")
            dma(ident[:], ident_d[:])
            epsbn = gp.tile([128, 1], f32, name="epsbn")
            nc.vector.memset(epsbn[:], 1e-5)

            def load_w(d, splits, M, nm):
                ts, r0 = [], 0
                for i, k in enumerate(splits):
                    t = gp.tile([k, M], f32, name=f"{nm}_{i}")
                    dma(t[:], d[r0:r0 + k, :])
                    ts.append(t)
                    r0 += k
                return ts

            w00 = load_w(w00_d, [64, 128], 128, "w00")
            w01 = load_w(w01_d, [128], 128, "w01")
            w10 = load_w(w10_d, [128, 128, 128], 256, "w10")
            w11 = load_w(w11_d, [128, 128], 128, "w11")
            w20 = load_w(w20_d, [128] * 6, 256, "w20")
            w21 = load_w(w21_d, [128, 128], 256, "w21")
            gb = {}
            for nm, nh in [("g00", 1), ("b00", 1), ("g01", 1), ("b01", 1),
                           ("g10", 2), ("b10", 2), ("g11", 1), ("b11", 1),
                           ("g20", 2), ("b20", 2), ("g21", 2), ("b21", 2)]:
                t = gp.tile([128, nh], f32, name=f"t_{nm}")
                dma(t[:], gb_d[nm][:])
                gb[nm] = t

            def allreduce(st, ncols, nm):
                sin = dp.tile([128, ncols], f32, name=f"sin_{nm}")
                sout = dp.tile([128, ncols], f32, name=f"sout_{nm}")
                nc.gpsimd.dma_start(sin[:], st[:, 0:ncols])
                nc.gpsimd.collective_compute(
                    "AllReduce", OP.add,
                    replica_groups=[list(range(8))],
                    ins=[sin.opt()], outs=[sout.opt()],
                )
                nc.gpsimd.dma_start(st[:, 0:ncols], sout[:])

            def bn_finalize(st, nh, g_t, b_t, invn):
                res = []
                for h in range(nh):
                    mean = sc.tile([128, 1], f32, name="mean", tag="mean", bufs=2)
                    nc.vector.tensor_scalar(mean[:], st[:, 2 * h:2 * h + 1], invn, None, OP.mult)
                    msq = sc.tile([128, 1], f32, name="msq", tag="msq", bufs=2)
                    nc.vector.tensor_scalar(msq[:], st[:, 2 * h + 1:2 * h + 2], invn, None, OP.mult)
                    m2 = sc.tile([128, 1], f32, name="m2", tag="m2", bufs=2)
                    nc.scalar.activation(m2[:], mean[:], AF.Square, bias=0.0)
                    var = sc.tile([128, 1], f32, name="var", tag="var", bufs=2)
                    nc.vector.tensor_scalar(var[:], msq[:], m2[:, 0:1], None, OP.subtract)
                    sd = sc.tile([128, 1], f32, name="sd", tag="sd", bufs=2)
                    nc.scalar.activation(sd[:], var[:], AF.Sqrt, bias=epsbn[:, 0:1])
                    inv = sc.tile([128, 1], f32, name="inv", tag="inv", bufs=2)
                    nc.vector.reciprocal(inv[:], sd[:])
                    s_t = sc.tile([128, 1], f32, name="s_t", tag="s_t", bufs=4)
                    nc.vector.tensor_scalar(s_t[:], g_t[:, h:h + 1], inv[:, 0:1], None, OP.mult)
                    ms = sc.tile([128, 1], f32, name="ms", tag="ms", bufs=2)
                    nc.vector.tensor_scalar(ms[:], mean[:], s_t[:, 0:1], None, OP.mult)
                    bb = sc.tile([128, 1], f32, name="bb", tag="bb", bufs=4)
                    nc.vector.tensor_scalar(bb[:], b_t[:, h:h + 1], ms[:, 0:1], None, OP.subtract)
                    res.append((s_t, bb))
                return res

            def conv_bn_layer(lp, xparts, wts, nh, n, g_t, b_t, invn, ydst, rdst, nm):
                nchunks = max(1, n // 512)
                cw = n // nchunks
                syp = [lp.tile([128, nchunks], f32, name=f"syp{h}{nm}") for h in range(nh)]
                sy2p = [lp.tile([128, nchunks], f32, name=f"sy2p{h}{nm}") for h in range(nh)]
                nk = len(xparts)
                for c in range(nchunks):
                    cs = slice(c * cw, (c + 1) * cw)
                    for h in range(nh):
                        pt = pp.tile([128, cw], f32, name="cv", tag="cv", bufs=2)
                        for ki, xt in enumerate(xparts):
                            nc.tensor.matmul(pt[:], wts[ki][:, h * 128:(h + 1) * 128],
                                             xt[:, cs], start=(ki == 0), stop=(ki == nk - 1))
                        nc.scalar.activation(ydst[h][:, cs], pt[:], AF.Copy, bias=0.0,
                                             accum_out=syp[h][:, c:c + 1])
                        sq = sc.tile([128, cw], f32, name="sq", tag="sq", bufs=2)
                        nc.scalar.activation(sq[:], pt[:], AF.Square, bias=0.0,
                                             accum_out=sy2p[h][:, c:c + 1])
                st = lp.tile([128, 2 * nh], f32, name=f"st{nm}")
                for h in range(nh):
                    nc.vector.tensor_reduce(st[:, 2 * h:2 * h + 1], syp[h][:], AX.X, OP.add)
                    nc.vector.tensor_reduce(st[:, 2 * h + 1:2 * h + 2], sy2p[h][:], AX.X, OP.add)
                allreduce(st, 2 * nh, nm)
                sbl = bn_finalize(st, nh, g_t, b_t, invn)
                for h in range(nh):
                    s_t, bb = sbl[h]
                    for c in range(nchunks):
                        cs = slice(c * cw, (c + 1) * cw)
                        nc.scalar.activation(rdst[h][:, cs], ydst[h][:, cs], AF.Relu,
                                             bias=bb[:, 0:1], scale=s_t[:, 0:1])

            def screen_interp(ntiles, m, Ct, qt, kt, us, table_d, wrap_d, xint):
                nmm = (m + 511) // 512
                for t in range(ntiles):
                    spm = pp.tile([128, m], f32, name="spm", tag="spm", bufs=1)
                    for k in range(nmm):
                        ks = slice(k * 512, min((k + 1) * 512, m))
                        nc.tensor.matmul(spm[:, ks], qt[:, t * 128:(t + 1) * 128],
                                         kt[:, ks], start=True, stop=True)
                    mx = sc.tile([128, 8], f32, name="mx", tag="mx", bufs=3)
                    nc.vector.max(mx[:], spm[:])
                    mi = sc.tile([128, 8], u16, name="mi", tag="mi", bufs=3)
                    nc.vector.max_index(mi[:], mx[:], spm[:])
                    t3 = sc.tile([128, 3], f32, name="t3", tag="t3", bufs=3)
                    nc.vector.tensor_scalar(t3[:], mx[:, 0:3], us[:, t:t + 1], EPS,
                                            OP.subtract, OP.subtract)
                    rc = sc.tile([128, 3], f32, name="rc", tag="rc", bufs=3)
                    nc.vector.reciprocal(rc[:], t3[:])
                    rs = sc.tile([128, 1], f32, name="rs", tag="rs", bufs=3)
                    nc.vector.tensor_reduce(rs[:], rc[:], AX.X, OP.add)
                    rsr = sc.tile([128, 1], f32, name="rsr", tag="rsr", bufs=3)
                    nc.vector.reciprocal(rsr[:], rs[:])
                    wn = sc.tile([128, 3], f32, name="wn", tag="wn", bufs=3)
                    nc.vector.tensor_scalar(wn[:], rc[:], rsr[:, 0:1], None, OP.mult)
                    for j in range(3):
                        dma(wrap_d[:, (t * 3 + j) * 8:(t * 3 + j) * 8 + 8].transpose([1, 0]),
                            mi[:, j:j + 1].bitcast(i16))
                    idxs = sc.tile([128, 24], i16, name="idxs", tag="idxs", bufs=2)
                    for g8 in range(8):
                        dma(idxs[g8 * 16:(g8 + 1) * 16, :], wrap_d[:, t * 24:(t + 1) * 24])
                    gt = sc.tile([128, 3, Ct], f32, name="gt", tag="gt", bufs=2)
                    nc.gpsimd.dma_gather(gt[:], table_d[:], idxs[:], 384, 384, Ct)
                    Ds = []
                    for j in range(3):
                        Dj = sc.tile([128, 128], f32, name=f"dg{j}", tag=f"dg{j}", bufs=2)
                        nc.scalar.activation(Dj[:], ident[:], AF.Copy, bias=0.0,
                                             scale=wn[:, j:j + 1])
                        Ds.append(Dj)
                    for h in range(Ct // 128):
                        ip = pp.tile([128, 128], f32, name="ip", tag="ip", bufs=2)
                        for j in range(3):
                            nc.tensor.matmul(ip[:], gt[:, j, h * 128:(h + 1) * 128],
                                             Ds[j][:], start=(j == 0), stop=(j == 2))
                        nc.scalar.activation(xint[h][:, t * 128:(t + 1) * 128], ip[:],
                                             AF.Copy, bias=0.0)

            def store_pm(src, dst_d, row0, col0):
                tp = pp.tile([128, 128], f32, name="tp", tag="ip", bufs=2)
                nc.tensor.transpose(tp[:], src, ident[:])
                tb = sc.tile([128, 128], f32, name="tb", tag="tb", bufs=2)
                nc.scalar.activation(tb[:], tp[:], AF.Copy, bias=0.0)
                dma(dst_d[row0:row0 + 128, col0:col0 + 128], tb[:])

            # ---------------- FP2 ----------------
            with tc.tile_pool(name="l2", bufs=1) as lp:
                qt = lp.tile([4, N2], f32, name="qt2t")
                dma(qt[:], qt2_d[:])
                kt = lp.tile([4, N3], f32, name="kt2t")
                dma(kt[:], kt2_d[:])
                us = lp.tile([128, 4], f32, name="us2t")
                dma(us[:], us2_d[:])
                f2a = lp.tile([128, N2], f32, name="f2a")
                dma(f2a[:], f2_d[0:128, :])
                f2b = lp.tile([128, N2], f32, name="f2b")
                dma(f2b[:], f2_d[128:256, :])
                xi = [lp.tile([128, N2], f32, name=f"xi2_{h}") for h in range(4)]
                ya = [lp.tile([128, N2], f32, name=f"ya2_{h}") for h in range(2)]
                screen_interp(4, N3, 512, qt, kt, us, f3pm_d, wrap_ds["s2"], xi)
                conv_bn_layer(lp, [f2a, f2b, xi[0], xi[1], xi[2], xi[3]], w20, 2, N2,
                              gb["g20"], gb["b20"], 1.0 / 4096.0, ya, [xi[0], xi[1]], "l2a")
                conv_bn_layer(lp, [xi[0], xi[1]], w21, 2, N2,
                              gb["g21"], gb["b21"], 1.0 / 4096.0, ya, [xi[2], xi[3]], "l2b")
                for h in range(2):
                    for t in range(4):
                        store_pm(xi[2 + h][:, t * 128:(t + 1) * 128], f2pm_d, t * 128, h * 128)

            # ---------------- FP1 ----------------
            with tc.tile_pool(name="l1", bufs=1) as lp:
                qt = lp.tile([4, N1], f32, name="qt1t")
                dma(qt[:], qt1_d[:])
                kt = lp.tile([4, N2], f32, name="kt1t")
                dma(kt[:], kt1_d[:])
                us = lp.tile([128, 16], f32, name="us1t")
                dma(us[:], us1_d[:])
                f1t = lp.tile([128, N1], f32, name="f1t")
                dma(f1t[:], f1_d[:])
                xi = [lp.tile([128, N1], f32, name=f"xi1_{h}") for h in range(2)]
                ya = [lp.tile([128, N1], f32, name=f"ya1_{h}") for h in range(2)]
                screen_interp(16, N2, 256, qt, kt, us, f2pm_d, wrap_ds["s1"], xi)
                conv_bn_layer(lp, [f1t, xi[0], xi[1]], w10, 2, N1,
                              gb["g10"], gb["b10"], 1.0 / 16384.0, ya, [xi[0], xi[1]], "l1a")
                conv_bn_layer(lp, [xi[0], xi[1]], w11, 1, N1,
                              gb["g11"], gb["b11"], 1.0 / 16384.0, [ya[0]], [ya[1]], "l1b")
                for t in range(16):
                    store_pm(ya[1][:, t * 128:(t + 1) * 128], f1pm_d, t * 128, 0)

            # ---------------- FP0 ----------------
            with tc.tile_pool(name="l0", bufs=1) as lp:
                qt = lp.tile([4, N0], f32, name="qt0t")
                dma(qt[:], qt0_d[:])
                kt = lp.tile([4, N1], f32, name="kt0t")
                dma(kt[:], kt0_d[:])
                us = lp.tile([128, 64], f32, name="us0t")
                dma(us[:], us0_d[:])
                f0t = lp.tile([64, N0], f32, name="f0t")
                dma(f0t[:], f0_d[:])
                xi0 = lp.tile([128, N0], f32, name="xi0")
                y0 = lp.tile([128, N0], f32, name="y0")
                screen_interp(64, N1, 128, qt, kt, us, f1pm_d, wrap_ds["s0"], [xi0])
                conv_bn_layer(lp, [f0t, xi0], w00, 1, N0,
                              gb["g00"], gb["b00"], 1.0 / 65536.0, [y0], [xi0], "l0a")
                conv_bn_layer(lp, [xi0], w01, 1, N0,
                              gb["g01"], gb["b01"], 1.0 / 65536.0, [y0], [xi0], "l0b")
                dma(out_d[:], xi0[:])

    nc.compile()
    return nc


def _prep_maps(inputs):
    def f32c(a):
        return np.ascontiguousarray(np.asarray(a, dtype=np.float32))

    def col(v, nh):
        return np.ascontiguousarray(np.asarray(v, np.float32).reshape(nh, 128).T)

    shared = {
        "w00": f32c(np.asarray(inputs["w00"]).T),
        "w01": f32c(np.asarray(inputs["w01"]).T),
        "w10": f32c(np.asarray(inputs["w10"]).T),
        "w11": f32c(np.asarray(inputs["w11"]).T),
        "w20": f32c(np.asarray(inputs["w20"]).T),
        "w21": f32c(np.asarray(inputs["w21"]).T),
    }
    for nm, nh in [("g00", 1), ("b00", 1), ("g01", 1), ("b01", 1),
                   ("g10", 2), ("b10", 2), ("g11", 1), ("b11", 1),
                   ("g20", 2), ("b20", 2), ("g21", 2), ("b21", 2)]:
        shared[nm] = col(inputs[nm], nh)

    def qk(q, k):
        qt = np.empty((4, q.shape[0]), np.float32)
        qt[0:3] = 2.0 * q.T
        qt[3] = 1.0
        kt = np.empty((4, k.shape[0]), np.float32)
        kt[0:3] = k.T
        kt[3] = -(k * k).sum(1)
        return np.ascontiguousarray(qt), np.ascontiguousarray(kt)

    def usq(q, nt):
        return np.ascontiguousarray((q * q).sum(1).astype(np.float32).reshape(nt, 128).T)

    xyz = [np.asarray(inputs[f"xyz{i}"], np.float32) for i in range(4)]
    maps = []
    for b in range(B):
        m = dict(shared)
        m["qt0"], m["kt0"] = qk(xyz[0][b], xyz[1][b])
        m["us0"] = usq(xyz[0][b], 64)
        m["qt1"], m["kt1"] = qk(xyz[1][b], xyz[2][b])
        m["us1"] = usq(xyz[1][b], 16)
        m["qt2"], m["kt2"] = qk(xyz[2][b], xyz[3][b])
        m["us2"] = usq(xyz[2][b], 4)
        m["f0"] = f32c(inputs["f0"][b])
        m["f1"] = f32c(inputs["f1"][b])
        m["f2"] = f32c(inputs["f2"][b])
        m["f3pm"] = f32c(np.asarray(inputs["f3"][b]).T)
        maps.append(m)
    return maps


def kernel(**inputs):
    from concourse.bass_utils import run_bass_kernel_spmd

    if "nc" not in _CTX:
        _CTX["nc"] = _build()
    nc = _CTX["nc"]
    maps = _prep_maps(inputs)
    res = run_bass_kernel_spmd(nc, maps, list(range(B)))
    return np.stack([res.results[b]["out"] for b in range(B)], axis=0)


# revision 4
# speedup vs baseline: 1.0613x; 1.0613x over previous
import sys

sys.path.insert(0, "/opt/trn_rl_repo")
import numpy as np

_CTX = {}

B = 8
N0, N1, N2, N3 = 8192, 2048, 512, 128
EPS = 1e-8


def _build():
    from concourse import bacc, tile
    import concourse.mybir as mybir

    f32 = mybir.dt.float32
    i16 = mybir.dt.int16
    u16 = mybir.dt.uint16
    AF = mybir.ActivationFunctionType
    OP = mybir.AluOpType
    AX = mybir.AxisListType

    nc = bacc.Bacc("TRN2", target_bir_lowering=False, debug=False, num_devices=8)

    qt0_d = nc.dram_tensor("qt0", [4, N0], f32, kind="ExternalInput")
    kt0_d = nc.dram_tensor("kt0", [4, N1], f32, kind="ExternalInput")
    us0_d = nc.dram_tensor("us0", [128, 64], f32, kind="ExternalInput")
    qt1_d = nc.dram_tensor("qt1", [4, N1], f32, kind="ExternalInput")
    kt1_d = nc.dram_tensor("kt1", [4, N2], f32, kind="ExternalInput")
    us1_d = nc.dram_tensor("us1", [128, 16], f32, kind="ExternalInput")
    qt2_d = nc.dram_tensor("qt2", [4, N2], f32, kind="ExternalInput")
    kt2_d = nc.dram_tensor("kt2", [4, N3], f32, kind="ExternalInput")
    us2_d = nc.dram_tensor("us2", [128, 4], f32, kind="ExternalInput")
    f0_d = nc.dram_tensor("f0", [64, N0], f32, kind="ExternalInput")
    f1_d = nc.dram_tensor("f1", [128, N1], f32, kind="ExternalInput")
    f2_d = nc.dram_tensor("f2", [256, N2], f32, kind="ExternalInput")
    f3pm_d = nc.dram_tensor("f3pm", [N3, 512], f32, kind="ExternalInput")
    w00_d = nc.dram_tensor("w00", [192, 128], f32, kind="ExternalInput")
    w01_d = nc.dram_tensor("w01", [128, 128], f32, kind="ExternalInput")
    w10_d = nc.dram_tensor("w10", [384, 256], f32, kind="ExternalInput")
    w11_d = nc.dram_tensor("w11", [256, 128], f32, kind="ExternalInput")
    w20_d = nc.dram_tensor("w20", [768, 256], f32, kind="ExternalInput")
    w21_d = nc.dram_tensor("w21", [256, 256], f32, kind="ExternalInput")
    gb_d = {}
    for nm, nh in [("g00", 1), ("b00", 1), ("g01", 1), ("b01", 1),
                   ("g10", 2), ("b10", 2), ("g11", 1), ("b11", 1),
                   ("g20", 2), ("b20", 2), ("g21", 2), ("b21", 2)]:
        gb_d[nm] = nc.dram_tensor(nm, [128, nh], f32, kind="ExternalInput")
    out_d = nc.dram_tensor("out", [128, N0], f32, kind="ExternalOutput")

    f2pm_d = nc.dram_tensor("f2pm", [N2, 256], f32, kind="Internal")
    f1pm_d = nc.dram_tensor("f1pm", [N1, 128], f32, kind="Internal")
    wrap_ds = {
        "s2": nc.dram_tensor("wrap2", [16, 24 * 4], i16, kind="Internal"),
        "s1": nc.dram_tensor("wrap1", [16, 24 * 16], i16, kind="Internal"),
        "s0": nc.dram_tensor("wrap0", [16, 24 * 64], i16, kind="Internal"),
    }
    ident_d = nc.inline_tensor(np.eye(128, dtype=np.float32), "identnp")

    with tile.TileContext(nc) as tc:
        with tc.tile_pool(name="glob", bufs=1) as gp, \
             tc.tile_pool(name="scr", bufs=1) as sc, \
             tc.tile_pool(name="ps", space="PSUM", bufs=1) as pp, \
             tc.tile_pool(name="dr", space="DRAM", bufs=1) as dp:

            def dma(dst, src):
                nc.sync.dma_start(dst, src)

            ident = gp.tile([128, 128], f32, name="ident")
            dma(ident[:], ident_d[:])
            epsbn = gp.tile([128, 1], f32, name="epsbn")
            nc.vector.memset(epsbn[:], 1e-5)

            def load_w(d, splits, M, nm):
                ts, r0 = [], 0
                for i, k in enumerate(splits):
                    t = gp.tile([k, M], f32, name=f"{nm}_{i}")
                    dma(t[:], d[r0:r0 + k, :])
                    ts.append(t)
                    r0 += k
                return ts

            w00 = load_w(w00_d, [64, 128], 128, "w00")
            w01 = load_w(w01_d, [128], 128, "w01")
            w10 = load_w(w10_d, [128, 128, 128], 256, "w10")
            w11 = load_w(w11_d, [128, 128], 128, "w11")
            w20 = load_w(w20_d, [128] * 6, 256, "w20")
            w21 = load_w(w21_d, [128, 128], 256, "w21")
            gb = {}
            for nm, nh in [("g00", 1), ("b00", 1), ("g01", 1), ("b01", 1),
                           ("g10", 2), ("b10", 2), ("g11", 1), ("b11", 1),
                           ("g20", 2), ("b20", 2), ("g21", 2), ("b21", 2)]:
                t = gp.tile([128, nh], f32, name=f"t_{nm}")
                dma(t[:], gb_d[nm][:])
                gb[nm] = t

            def allreduce(st, ncols, nm):
                sin = dp.tile([128, ncols], f32, name=f"sin_{nm}")
                sout = dp.tile([128, ncols], f32, name=f"sout_{nm}")
                nc.gpsimd.dma_start(sin[:], st[:, 0:ncols])
                nc.gpsimd.collective_compute(
                    "AllReduce", OP.add,
                    replica_groups=[list(range(8))],
                    ins=[sin.opt()], outs=[sout.opt()],
                )
                nc.gpsimd.dma_start(st[:, 0:ncols], sout[:])

            def bn_finalize(st, nh, g_t, b_t, invn):
                res = []
                for h in range(nh):
                    mean = sc.tile([128, 1], f32, name="mean", tag="mean", bufs=2)
                    nc.vector.tensor_scalar(mean[:], st[:, 2 * h:2 * h + 1], invn, None, OP.mult)
                    msq = sc.tile([128, 1], f32, name="msq", tag="msq", bufs=2)
                    nc.vector.tensor_scalar(msq[:], st[:, 2 * h + 1:2 * h + 2], invn, None, OP.mult)
                    m2 = sc.tile([128, 1], f32, name="m2", tag="m2", bufs=2)
                    nc.scalar.activation(m2[:], mean[:], AF.Square, bias=0.0)
                    var = sc.tile([128, 1], f32, name="var", tag="var", bufs=2)
                    nc.vector.tensor_scalar(var[:], msq[:], m2[:, 0:1], None, OP.subtract)
                    sd = sc.tile([128, 1], f32, name="sd", tag="sd", bufs=2)
                    nc.scalar.activation(sd[:], var[:], AF.Sqrt, bias=epsbn[:, 0:1])
                    inv = sc.tile([128, 1], f32, name="inv", tag="inv", bufs=2)
                    nc.vector.reciprocal(inv[:], sd[:])
                    s_t = sc.tile([128, 1], f32, name="s_t", tag="s_t", bufs=4)
                    nc.vector.tensor_scalar(s_t[:], g_t[:, h:h + 1], inv[:, 0:1], None, OP.mult)
                    ms = sc.tile([128, 1], f32, name="ms", tag="ms", bufs=2)
                    nc.vector.tensor_scalar(ms[:], mean[:], s_t[:, 0:1], None, OP.mult)
                    bb = sc.tile([128, 1], f32, name="bb", tag="bb", bufs=4)
                    nc.vector.tensor_scalar(bb[:], b_t[:, h:h + 1], ms[:, 0:1], None, OP.subtract)
                    res.append((s_t, bb))
                return res

            def conv_bn_layer(lp, xparts, wts, nh, n, g_t, b_t, invn, ydst, rdst, nm):
                nchunks = max(1, n // 512)
                cw = n // nchunks
                syp = [lp.tile([128, nchunks], f32, name=f"syp{h}{nm}") for h in range(nh)]
                sy2p = [lp.tile([128, nchunks], f32, name=f"sy2p{h}{nm}") for h in range(nh)]
                nk = len(xparts)
                for c in range(nchunks):
                    cs = slice(c * cw, (c + 1) * cw)
                    for h in range(nh):
                        pt = pp.tile([128, cw], f32, name="cv", tag="cv", bufs=2)
                        for ki, xt in enumerate(xparts):
                            nc.tensor.matmul(pt[:], wts[ki][:, h * 128:(h + 1) * 128],
                                             xt[:, cs], start=(ki == 0), stop=(ki == nk - 1))
                        nc.scalar.activation(ydst[h][:, cs], pt[:], AF.Copy, bias=0.0,
                                             accum_out=syp[h][:, c:c + 1])
                        sq = sc.tile([128, cw], f32, name="sq", tag="sq", bufs=2)
                        nc.scalar.activation(sq[:], pt[:], AF.Square, bias=0.0,
                                             accum_out=sy2p[h][:, c:c + 1])
                st = lp.tile([128, 2 * nh], f32, name=f"st{nm}")
                for h in range(nh):
                    nc.vector.tensor_reduce(st[:, 2 * h:2 * h + 1], syp[h][:], AX.X, OP.add)
                    nc.vector.tensor_reduce(st[:, 2 * h + 1:2 * h + 2], sy2p[h][:], AX.X, OP.add)
                allreduce(st, 2 * nh, nm)
                sbl = bn_finalize(st, nh, g_t, b_t, invn)
                for h in range(nh):
                    s_t, bb = sbl[h]
                    for c in range(nchunks):
                        cs = slice(c * cw, (c + 1) * cw)
                        nc.scalar.activation(rdst[h][:, cs], ydst[h][:, cs], AF.Relu,
                                             bias=bb[:, 0:1], scale=s_t[:, 0:1])

            def screen_interp(ntiles, m, Ct, qt, kt, us, table_d, wrap_d, xint):
                nmm = (m + 511) // 512
                for t in range(ntiles):
                    spm = pp.tile([128, m], f32, name="spm", tag="spm", bufs=1)
                    for k in range(nmm):
                        ks = slice(k * 512, min((k + 1) * 512, m))
                        nc.tensor.matmul(spm[:, ks], qt[:, t * 128:(t + 1) * 128],
                                         kt[:, ks], start=True, stop=True)
                    mx = sc.tile([128, 8], f32, name="mx", tag="mx", bufs=3)
                    nc.vector.max(mx[:], spm[:])
                    mi = sc.tile([128, 8], u16, name="mi", tag="mi", bufs=3)
                    nc.vector.max_index(mi[:], mx[:], spm[:])
                    t3 = sc.tile([128, 3], f32, name="t3", tag="t3", bufs=3)
                    nc.vector.tensor_scalar(t3[:], mx[:, 0:3], us[:, t:t + 1], EPS,
                                            OP.subtract, OP.subtract)
                    rc = sc.tile([128, 3], f32, name="rc", tag="rc", bufs=3)
                    nc.vector.reciprocal(rc[:], t3[:])
                    rs = sc.tile([128, 1], f32, name="rs", tag="rs", bufs=3)
                    nc.vector.tensor_reduce(rs[:], rc[:], AX.X, OP.add)
                    rsr = sc.tile([128, 1], f32, name="rsr", tag="rsr", bufs=3)
                    nc.vector.reciprocal(rsr[:], rs[:])
                    wn = sc.tile([128, 3], f32, name="wn", tag="wn", bufs=3)
                    nc.vector.tensor_scalar(wn[:], rc[:], rsr[:, 0:1], None, OP.mult)
                    for j in range(3):
                        dma(wrap_d[:, (t * 3 + j) * 8:(t * 3 + j) * 8 + 8].transpose([1, 0]),
                            mi[:, j:j + 1].bitcast(i16))
                    idxs = sc.tile([128, 24], i16, name="idxs", tag="idxs", bufs=2)
                    for g8 in range(8):
                        dma(idxs[g8 * 16:(g8 + 1) * 16, :], wrap_d[:, t * 24:(t + 1) * 24])
                    gt = sc.tile([128, 3, Ct], f32, name="gt", tag="gt", bufs=2)
                    nc.gpsimd.dma_gather(gt[:], table_d[:], idxs[:], 384, 384, Ct)
                    Ds = []
                    for j in range(3):
                        Dj = sc.tile([128, 128], f32, name=f"dg{j}", tag=f"dg{j}", bufs=2)
                        nc.scalar.activation(Dj[:], ident[:], AF.Copy, bias=0.0,
                                             scale=wn[:, j:j + 1])
                        Ds.append(Dj)
                    for h in range(Ct // 128):
                        ip = pp.tile([128, 128], f32, name="ip", tag="ip", bufs=2)
                        for j in range(3):
                            nc.tensor.matmul(ip[:], gt[:, j, h * 128:(h + 1) * 128],
                                             Ds[j][:], start=(j == 0), stop=(j == 2))
                        nc.scalar.activation(xint[h][:, t * 128:(t + 1) * 128], ip[:],
                                             AF.Copy, bias=0.0)

            def store_pm(src, dst_d, row0, col0):
                tp = pp.tile([128, 128], f32, name="tp", tag="ip", bufs=2)
                nc.tensor.transpose(tp[:], src, ident[:])
                tb = sc.tile([128, 128], f32, name="tb", tag="tb", bufs=2)
                nc.scalar.activation(tb[:], tp[:], AF.Copy, bias=0.0)
                dma(dst_d[row0:row0 + 128, col0:col0 + 128], tb[:])

            # ---------------- FP2 ----------------
            with tc.tile_pool(name="l2", bufs=1) as lp:
                qt = lp.tile([4, N2], f32, name="qt2t")
                dma(qt[:], qt2_d[:])
                kt = lp.tile([4, N3], f32, name="kt2t")
                dma(kt[:], kt2_d[:])
                us = lp.tile([128, 4], f32, name="us2t")
                dma(us[:], us2_d[:])
                f2a = lp.tile([128, N2], f32, name="f2a")
                dma(f2a[:], f2_d[0:128, :])
                f2b = lp.tile([128, N2], f32, name="f2b")
                dma(f2b[:], f2_d[128:256, :])
                xi = [lp.tile([128, N2], f32, name=f"xi2_{h}") for h in range(4)]
                ya = [lp.tile([128, N2], f32, name=f"ya2_{h}") for h in range(2)]
                screen_interp(4, N3, 512, qt, kt, us, f3pm_d, wrap_ds["s2"], xi)
                conv_bn_layer(lp, [f2a, f2b, xi[0], xi[1], xi[2], xi[3]], w20, 2, N2,
                              gb["g20"], gb["b20"], 1.0 / 4096.0, ya, [xi[0], xi[1]], "l2a")
                conv_bn_layer(lp, [xi[0], xi[1]], w21, 2, N2,
                              gb["g21"], gb["b21"], 1.0 / 4096.0, ya, [xi[2], xi[3]], "l2b")
                for h in range(2):
                    for t in range(4):
                        store_pm(xi[2 + h][:, t * 128:(t + 1) * 128], f2pm_d, t * 128, h * 128)

            # ---------------- FP1 ----------------
            with tc.tile_pool(name="l1", bufs=1) as lp:
                qt = lp.tile([4, N1], f32, name="qt1t")
                dma(qt[:], qt1_d[:])
                kt = lp.tile([4, N2], f32, name="kt1t")
                dma(kt[:], kt1_d[:])
                us = lp.tile([128, 16], f32, name="us1t")
                dma(us[:], us1_d[:])
                f1t = lp.tile([128, N1], f32, name="f1t")
                dma(f1t[:], f1_d[:])
                xi = [lp.tile([128, N1], f32, name=f"xi1_{h}") for h in range(2)]
                ya = [lp.tile([128, N1], f32, name=f"ya1_{h}") for h in range(2)]
                screen_interp(16, N2, 256, qt, kt, us, f2pm_d, wrap_ds["s1"], xi)
                conv_bn_layer(lp, [f1t, xi[0], xi[1]], w10, 2, N1,
                              gb["g10"], gb["b10"], 1.0 / 16384.0, ya, [xi[0], xi[1]], "l1a")
                conv_bn_layer(lp, [xi[0], xi[1]], w11, 1, N1,
                              gb["g11"], gb["b11"], 1.0 / 16384.0, [ya[0]], [ya[1]], "l1b")
                for t in range(16):
                    store_pm(ya[1][:, t * 128:(t + 1) * 128], f1pm_d, t * 128, 0)

            # ---------------- FP0 ----------------
            with tc.tile_pool(name="l0", bufs=1) as lp:
                qt = lp.tile([4, N0], f32, name="qt0t")
                dma(qt[:], qt0_d[:])
                kt = lp.tile([4, N1], f32, name="kt0t")
                dma(kt[:], kt0_d[:])
                us = lp.tile([128, 64], f32, name="us0t")
                dma(us[:], us0_d[:])
                f0t = lp.tile([64, N0], f32, name="f0t")
                dma(f0t[:], f0_d[:])
                xi0 = lp.tile([128, N0], f32, name="xi0")
                y0 = lp.tile([128, N0], f32, name="y0")
                screen_interp(64, N1, 128, qt, kt, us, f1pm_d, wrap_ds["s0"], [xi0])
                conv_bn_layer(lp, [f0t, xi0], w00, 1, N0,
                              gb["g00"], gb["b00"], 1.0 / 65536.0, [y0], [xi0], "l0a")
                conv_bn_layer(lp, [xi0], w01, 1, N0,
                              gb["g01"], gb["b01"], 1.0 / 65536.0, [y0], [xi0], "l0b")
                dma(out_d[:], xi0[:])

    nc.compile()
    return nc


def _prep_maps(inputs):
    def f32c(a):
        return np.ascontiguousarray(np.asarray(a, dtype=np.float32))

    def col(v, nh):
        return np.ascontiguousarray(np.asarray(v, np.float32).reshape(nh, 128).T)

    shared = {
        "w00": f32c(np.asarray(inputs["w00"]).T),
        "w01": f32c(np.asarray(inputs["w01"]).T),
        "w10": f32c(np.asarray(inputs["w10"]).T),
        "w11": f32c(np.asarray(inputs["w11"]).T),
        "w20": f32c(np.asarray(inputs["w20"]).T),
        "w21": f32c(np.asarray(inputs["w21"]).T),
    }
    for nm, nh in [("g00", 1), ("b00", 1), ("g01", 1), ("b01", 1),
                   ("g10", 2), ("b10", 2), ("g11", 1), ("b11", 1),
                   ("g20", 2), ("b20", 2), ("g21", 2), ("b21", 2)]:
        shared[nm] = col(inputs[nm], nh)

    def qk(q, k):
        q = q - 0.5
        k = k - 0.5
        qt = np.empty((4, q.shape[0]), np.float32)
        qt[0:3] = 2.0 * q.T
        qt[3] = 1.0
        kt = np.empty((4, k.shape[0]), np.float32)
        kt[0:3] = k.T
        kt[3] = -(k * k).sum(1)
        return np.ascontiguousarray(qt), np.ascontiguousarray(kt)

    def usq(q, nt):
        q = q - 0.5
        return np.ascontiguousarray((q * q).sum(1).astype(np.float32).reshape(nt, 128).T)

    xyz = [np.asarray(inputs[f"xyz{i}"], np.float32) for i in range(4)]
    maps = []
    for b in range(B):
        m = dict(shared)
        m["qt0"], m["kt0"] = qk(xyz[0][b], xyz[1][b])
        m["us0"] = usq(xyz[0][b], 64)
        m["qt1"], m["kt1"] = qk(xyz[1][b], xyz[2][b])
        m["us1"] = usq(xyz[1][b], 16)
        m["qt2"], m["kt2"] = qk(xyz[2][b], xyz[3][b])
        m["us2"] = usq(xyz[2][b], 4)
        m["f0"] = f32c(inputs["f0"][b])
        m["f1"] = f32c(inputs["f1"][b])
        m["f2"] = f32c(inputs["f2"][b])
        m["f3pm"] = f32c(np.asarray(inputs["f3"][b]).T)
        maps.append(m)
    return maps


def kernel(**inputs):
    from concourse.bass_utils import run_bass_kernel_spmd

    if "nc" not in _CTX:
        _CTX["nc"] = _build()
    nc = _CTX["nc"]
    maps = _prep_maps(inputs)
    res = run_bass_kernel_spmd(nc, maps, list(range(B)))
    return np.stack([res.results[b]["out"] for b in range(B)], axis=0)
